# revision 6
# baseline (speedup 1.0000x reference)
"""Trainium2 kernel for nn_ConvolutionFeatureModel.

Computes out = relu(||w_n - x_m||_2 / sqrt(512)) for x (8192, 512) and
weight (4096, 512), out (8192, 4096), all fp32.

Math:  sq_dist[m,n] = ||x_m||^2 + ||w_n||^2 - 2 x_m.w_n   (a GEMM + epilogue)
       out = sqrt(sq_dist / 512)            (relu is a no-op: sqrt >= 0)

Sharding: 8 cores as 4 batch-groups x 2 width-groups.  Per core:
x-shard (2048, 512), w-shard (2048, 512) -> out block (2048, 2048).

Per-core pipeline (Tile framework):
 - gpsimd DMA loads with fp32->bf16 cast into [128, 16, 512] stages.
 - TensorE transposes (via identity) produce xT/wT [128k, 4chunk, 2048] bf16.
 - ||w||^2/512 broadcast to all partitions via ones-matmul over squared wT.
 - ||x||^2 via ScalarE Square activation with accum_out (per-partition col).
 - Main GEMM: 8 matmuls per [128, 1024] PSUM tile (4 k-chunks x 2 n-subtiles).
 - Epilogue: DVE  t1 = psum * (-2/512) + wsq_bcast   (scalar_tensor_tensor)
             ACT  o  = Sqrt(t1 + xsq/512[bias])      (activation)
 - HWDGE (sync) DMA stores fp32 out tiles.
"""

import numpy as np

import concourse.bass as bass
import concourse.mybir as mybir
import concourse.tile as tile
from concourse import bacc
from concourse.masks import make_identity

P = 128          # partitions
K = 512          # contraction (input_dim)
KC = K // P      # 4 k-chunks
M = 2048         # batch rows per core   (8192 / 4 batch groups)
N = 2048         # width cols per core   (4096 / 2 width groups)
MT = M // P      # 16 m-tiles
NH = N // 1024   # 2 n-halves (one [128, 1024] psum tile each)
R512 = 1.0 / 512.0   # 1/SCALE^2 (SCALE = sqrt(512))

BF16 = mybir.dt.bfloat16
F32 = mybir.dt.float32


def build_nc(repeats=1):
    nc = bacc.Bacc("TRN2", target_bir_lowering=False)
    x_d = nc.dram_tensor("x", [M, K], F32, kind="ExternalInput")
    w_d = nc.dram_tensor("w", [N, K], F32, kind="ExternalInput")
    o_d = nc.dram_tensor("out", [M, N], F32, kind="ExternalOutput")

    AL = mybir.AluOpType
    with tile.TileContext(nc) as tc:
      for _rep in range(repeats):
        with (
            tc.tile_pool(name="big", bufs=1) as big,
            tc.tile_pool(name="tp_ps", bufs=2, space=bass.MemorySpace.PSUM) as tp_ps,
            tc.tile_pool(name="mm_ps", bufs=3, space=bass.MemorySpace.PSUM) as mm_ps,
            tc.tile_pool(name="sqwp", bufs=2) as sqwp,
            tc.tile_pool(name="t1p", bufs=3) as t1p,
            tc.tile_pool(name="outp", bufs=3) as outp,
        ):
            ident = big.tile([P, P], BF16, tag="ident")
            make_identity(nc, ident[:, :])
            ones = big.tile([P, P], BF16, tag="ones")
            nc.gpsimd.memset(ones[:, :], 1.0)

            x_stage = big.tile([P, MT, K], BF16, tag="xs")   # [m_in_tile, t, k]
            w_stage = big.tile([P, MT, K], BF16, tag="ws")
            xT = big.tile([P, KC, M], BF16, tag="xT")        # [k_in_chunk, c, m]
            wT = big.tile([P, KC, N], BF16, tag="wT")
            wsq = big.tile([P, N], F32, tag="wsq")           # ||w||^2/512, bcast
            xsq = big.tile([P, MT], F32, tag="xsq")          # raw sum x^2
            xsq_s = big.tile([P, MT], F32, tag="xsqs")       # /512
            trash = big.tile([P, K], F32, tag="trash")

            x_r = x_d.rearrange("(t p) k -> p t k", p=P)
            w_r = w_d.rearrange("(t p) k -> p t k", p=P)
            # Loads with fp32->bf16 cast (SWDGE).  4 x 1 MiB reads each.
            for i in range(4):
                nc.gpsimd.dma_start(
                    out=w_stage[:, 4 * i : 4 * i + 4, :],
                    in_=w_r[:, 4 * i : 4 * i + 4, :],
                )
            for i in range(4):
                nc.gpsimd.dma_start(
                    out=x_stage[:, 4 * i : 4 * i + 4, :],
                    in_=x_r[:, 4 * i : 4 * i + 4, :],
                )

            def transpose_pair(stage, dst, t2):
                """Transpose stage tiles 2*t2, 2*t2+1 into dst[:, :, 256-col slice]."""
                ps = tp_ps.tile([P, KC, 2, P], BF16, tag="tps")
                for c in range(KC):
                    for tt in range(2):
                        t = 2 * t2 + tt
                        nc.tensor.transpose(
                            ps[:, c, tt, :],
                            stage[:, t, c * P : (c + 1) * P],
                            ident[:, :],
                        )
                dst_v = dst[:, :, 2 * t2 * P : (2 * t2 + 2) * P].rearrange(
                    "p c (tt j) -> p c tt j", j=P
                )
                nc.vector.tensor_copy(out=dst_v, in_=ps[:, :, :, :])

            # w first: wT feeds both the main GEMM and the wsq reduction.
            for t2 in range(MT // 2):
                transpose_pair(w_stage, wT, t2)
            for t2 in range(MT // 2):
                transpose_pair(x_stage, xT, t2)

            # ||x||^2 per m-tile (ScalarE Square + row accumulation).
            for t in range(MT):
                nc.scalar.activation(
                    out=trash[:, :],
                    in_=x_stage[:, t, :],
                    func=mybir.ActivationFunctionType.Square,
                    accum_out=xsq[:, t : t + 1],
                )
            nc.vector.tensor_scalar_mul(
                out=xsq_s[:, :], in0=xsq[:, :], scalar1=R512
            )

            # ||w||^2/512 broadcast to all partitions: ones-matmul over wT^2.
            wsq_ps = [
                mm_ps.tile([P, 1024], F32, tag="mm", name=f"wsq_ps{h}")
                for h in range(NH)
            ]
            for c in range(KC):
                sqw = sqwp.tile([P, N], BF16, tag="sqw")
                nc.vector.scalar_tensor_tensor(
                    out=sqw[:, :],
                    in0=wT[:, c, :],
                    scalar=R512,
                    in1=wT[:, c, :],
                    op0=AL.mult,
                    op1=AL.mult,
                )
                for q in range(N // 512):
                    h, s = divmod(q, 2)
                    nc.tensor.matmul(
                        wsq_ps[h][:, s * 512 : (s + 1) * 512],
                        ones[:, :],
                        sqw[:, q * 512 : (q + 1) * 512],
                        start=(c == 0),
                        stop=(c == KC - 1),
                    )
            for h in range(NH):
                nc.vector.tensor_copy(
                    out=wsq[:, h * 1024 : (h + 1) * 1024], in_=wsq_ps[h][:, :]
                )

            # Main loop: 16 m-tiles x 2 n-halves.
            for t in range(MT):
                for h in range(NH):
                    ps = mm_ps.tile([P, 1024], F32, tag="mm")
                    for c in range(KC):
                        for s in range(2):
                            n0 = h * 1024 + s * 512
                            nc.tensor.matmul(
                                ps[:, s * 512 : (s + 1) * 512],
                                xT[:, c, t * P : (t + 1) * P],
                                wT[:, c, n0 : n0 + 512],
                                start=(c == 0),
                                stop=(c == KC - 1),
                            )
                    t1 = t1p.tile([P, 1024], F32, tag="t1")
                    nc.vector.scalar_tensor_tensor(
                        out=t1[:, :],
                        in0=ps[:, :],
                        scalar=-2.0 * R512,
                        in1=wsq[:, h * 1024 : (h + 1) * 1024],
                        op0=AL.mult,
                        op1=AL.add,
                    )
                    o = outp.tile([P, 1024], F32, tag="o")
                    nc.scalar.activation(
                        out=o[:, :],
                        in_=t1[:, :],
                        func=mybir.ActivationFunctionType.Sqrt,
                        bias=xsq_s[:, t : t + 1],
                        scale=1.0,
                    )
                    nc.sync.dma_start(
                        out=o_d[t * P : (t + 1) * P, h * 1024 : (h + 1) * 1024],
                        in_=o[:, :],
                    )
    nc.compile()
    return nc


_NC_CACHE = None


def _get_nc():
    global _NC_CACHE
    if _NC_CACHE is None:
        _NC_CACHE = build_nc()
    return _NC_CACHE


def kernel(x, weight):
    from concourse.bass_utils import run_bass_kernel_spmd

    x = np.ascontiguousarray(np.asarray(x, dtype=np.float32))
    weight = np.ascontiguousarray(np.asarray(weight, dtype=np.float32))
    assert x.shape == (8192, 512) and weight.shape == (4096, 512)

    nc = _get_nc()
    in_maps = []
    for c in range(8):
        bg, wg = divmod(c, 2)
        in_maps.append(
            {
                "x": x[bg * M : (bg + 1) * M],
                "w": weight[wg * N : (wg + 1) * N],
            }
        )
    res = run_bass_kernel_spmd(nc, in_maps, core_ids=list(range(8)))
    out = np.empty((8192, 4096), dtype=np.float32)
    for c in range(8):
        bg, wg = divmod(c, 2)
        out[bg * M : (bg + 1) * M, wg * N : (wg + 1) * N] = res.results[c]["out"]
    return out


# revision 26
# speedup vs baseline: 379.3581x; 379.3581x over previous
"""Trainium2 kernel for nn_ConvolutionFeatureModel.

Computes out = relu(||w_n - x_m||_2 / sqrt(512)) for x (8192, 512) and
weight (4096, 512), out (8192, 4096), all fp32.

Math:  sq_dist[m,n] = ||x_m||^2 + ||w_n||^2 - 2 x_m.w_n   (a GEMM + epilogue)
       out = sqrt(sq_dist / 512)            (relu is a no-op: sqrt >= 0)

Sharding: 8 cores as 4 batch-groups x 2 width-groups.  Per core:
x-shard (2048, 512), w-shard (2048, 512) -> out block (2048, 2048).

Per-core pipeline (Tile framework):
 - gpsimd DMA loads with fp32->bf16 cast into [128, 16, 512] stages.
 - TensorE transposes (via identity) produce xT/wT [128k, 4chunk, 2048] bf16.
 - ||w||^2/512 broadcast to all partitions via ones-matmul over squared wT.
 - ||x||^2 via ScalarE Square activation with accum_out (per-partition col).
 - Main GEMM: 8 matmuls per [128, 1024] PSUM tile (4 k-chunks x 2 n-subtiles).
 - Epilogue: DVE  t1 = psum * (-2/512) + wsq_bcast   (scalar_tensor_tensor)
             ACT  o  = Sqrt(t1 + xsq/512[bias])      (activation)
 - HWDGE (sync) DMA stores fp32 out tiles.
"""

import numpy as np

import concourse.bass as bass
import concourse.mybir as mybir
import concourse.tile as tile
from concourse import bacc
from concourse.masks import make_identity

P = 128          # partitions
K = 512          # contraction (input_dim)
KC = K // P      # 4 k-chunks
M = 2048         # batch rows per core   (8192 / 4 batch groups)
N = 2048         # width cols per core   (4096 / 2 width groups)
MT = M // P      # 16 m-tiles
NH = N // 1024   # 2 n-halves (one [128, 1024] psum tile each)
R512 = 1.0 / 512.0   # 1/SCALE^2 (SCALE = sqrt(512))

BF16 = mybir.dt.bfloat16
F32 = mybir.dt.float32

# Tunables (swept via analyze/sweep scripts)
LOAD_BOUNDS = [0, 2, 6, 11, 16]
INTERLEAVE = True      # interleave w/x transpose pairs
LAST_SPLIT = True      # split final m-tile into 512-wide groups
MM_BUFS = 3            # psum slots on the "mm" tag (2 banks each)
T1_BUFS = 6
OUT_BUFS = 6
TP_SPLIT = True        # alternate transpose-copy between DVE and ACT
XSQ_GPSIMD = False     # GPSIMD lacks accum_out on HW (walrus rejects)
SQW_GPSIMD = False     # square wT on GPSIMD instead of DVE


def build_nc(repeats=1, level=5):
    nc = bacc.Bacc("TRN2", target_bir_lowering=False)
    x_d = nc.dram_tensor("x", [M, K], F32, kind="ExternalInput")
    w_d = nc.dram_tensor("w", [N, K], F32, kind="ExternalInput")
    o_d = nc.dram_tensor("out", [M, N], F32, kind="ExternalOutput")

    AL = mybir.AluOpType
    with tile.TileContext(nc) as tc:
      for _rep in range(repeats):
        with (
            tc.tile_pool(name="big", bufs=1) as big,
            tc.tile_pool(name="mm_ps", bufs=MM_BUFS, space=bass.MemorySpace.PSUM) as mm_ps,
            tc.tile_pool(name="tp_ps", bufs=2, space=bass.MemorySpace.PSUM) as tp_ps,
            tc.tile_pool(name="sqwp", bufs=2) as sqwp,
            tc.tile_pool(name="t1p", bufs=T1_BUFS) as t1p,
            tc.tile_pool(name="outp", bufs=OUT_BUFS) as outp,
        ):
            ident = big.tile([P, P], BF16, tag="ident")
            make_identity(nc, ident[:, :])
            ones = big.tile([P, P], BF16, tag="ones")
            nc.gpsimd.memset(ones[:, :], 1.0)

            x_stage = big.tile([P, MT, K], BF16, tag="xs")   # [m_in_tile, t, k]
            w_stage = big.tile([P, MT, K], BF16, tag="ws")
            xT = big.tile([P, KC, M], BF16, tag="xT")        # [k_in_chunk, c, m]
            wT = big.tile([P, KC, N], BF16, tag="wT")
            wsq = big.tile([P, N], F32, tag="wsq")           # ||w||^2/512, bcast
            xsq_s = big.tile([P, MT], F32, tag="xsqs")       # sum x^2 / 512
            trash = big.tile([P, K], F32, tag="trash")

            x_r = x_d.rearrange("(t p) k -> p t k", p=P)
            w_r = w_d.rearrange("(t p) k -> p t k", p=P)
            # Loads with fp32->bf16 cast (SWDGE), w/x interleaved, small
            # first chunks so transposes can start early.
            bounds = LOAD_BOUNDS
            for i in range(len(bounds) - 1):
                lo, hi = bounds[i], bounds[i + 1]
                nc.gpsimd.dma_start(
                    out=w_stage[:, lo:hi, :], in_=w_r[:, lo:hi, :]
                )
                nc.gpsimd.dma_start(
                    out=x_stage[:, lo:hi, :], in_=x_r[:, lo:hi, :]
                )

            if level < 2:
                continue

            def transpose_pair(stage, dst, t2, on_act=False):
                """Transpose stage tiles 2*t2, 2*t2+1 into dst[:, :, 256-col slice]."""
                ps = tp_ps.tile([P, KC, 2, P], BF16, tag="tps", name="tps")
                for c in range(KC):
                    for tt in range(2):
                        t = 2 * t2 + tt
                        nc.tensor.transpose(
                            ps[:, c, tt, :],
                            stage[:, t, c * P : (c + 1) * P],
                            ident[:, :],
                        )
                dst_v = dst[:, :, 2 * t2 * P : (2 * t2 + 2) * P].rearrange(
                    "p c (tt j) -> p c tt j", j=P
                )
                if on_act:
                    nc.scalar.copy(out=dst_v, in_=ps[:, :, :, :])
                else:
                    nc.vector.tensor_copy(out=dst_v, in_=ps[:, :, :, :])

            emit_math = level >= 3

            def emit_xsq(t):
                # ||x||^2 for x tile t (square + row-sum accumulate).
                if XSQ_GPSIMD:
                    nc.gpsimd.scalar_tensor_tensor(
                        out=trash[:, :],
                        in0=x_stage[:, t, :],
                        scalar=R512,
                        in1=x_stage[:, t, :],
                        op0=AL.mult,
                        op1=AL.mult,
                        accum_out=xsq_s[:, t : t + 1],
                    )
                else:
                    nc.scalar.activation(
                        out=trash[:, :],
                        in_=x_stage[:, t, :],
                        func=mybir.ActivationFunctionType.Square,
                        scale=0.04419417382415922,
                        accum_out=xsq_s[:, t : t + 1],
                    )

            def emit_wsq(h):
                wsq_ps = mm_ps.tile([P, 1024], F32, tag="mm", name="wsq_ps")
                for c in range(KC):
                    sqw = sqwp.tile([P, 1024], BF16, tag="sqw", name="sqw")
                    eng = nc.gpsimd if SQW_GPSIMD else nc.vector
                    eng.scalar_tensor_tensor(
                        out=sqw[:, :],
                        in0=wT[:, c, h * 1024 : (h + 1) * 1024],
                        scalar=R512,
                        in1=wT[:, c, h * 1024 : (h + 1) * 1024],
                        op0=AL.mult,
                        op1=AL.mult,
                    )
                    for s in range(2):
                        nc.tensor.matmul(
                            wsq_ps[:, s * 512 : (s + 1) * 512],
                            ones[:, :],
                            sqw[:, s * 512 : (s + 1) * 512],
                            start=(c == 0),
                            stop=(c == KC - 1),
                        )
                nc.vector.tensor_copy(
                    out=wsq[:, h * 1024 : (h + 1) * 1024], in_=wsq_ps[:, :]
                )

            def main_group(t, h, nw=1024):
                """One out-tile group: GEMM accumulate + epilogue + store."""
                ps = mm_ps.tile([P, nw], F32, tag="mm", name="ps")
                for c in range(KC):
                    for s in range(nw // 512):
                        nc.tensor.matmul(
                            ps[:, s * 512 : (s + 1) * 512],
                            xT[:, c, t * P : (t + 1) * P],
                            wT[:, c, h * 1024 + s * 512 : h * 1024 + (s + 1) * 512],
                            start=(c == 0),
                            stop=(c == KC - 1),
                        )
                if level < 4:
                    return
                t1 = t1p.tile([P, nw], F32, tag="t1", name="t1")
                nc.vector.scalar_tensor_tensor(
                    out=t1[:, :],
                    in0=ps[:, :],
                    scalar=-2.0 * R512,
                    in1=wsq[:, h * 1024 : h * 1024 + nw],
                    op0=AL.mult,
                    op1=AL.add,
                )
                if level < 5:
                    return
                o = outp.tile([P, nw], F32, tag="o", name="o")
                nc.scalar.activation(
                    out=o[:, :],
                    in_=t1[:, :],
                    func=mybir.ActivationFunctionType.Sqrt,
                    bias=xsq_s[:, t : t + 1],
                    scale=1.0,
                )
                nc.sync.dma_start(
                    out=o_d[t * P : (t + 1) * P, h * 1024 : h * 1024 + nw],
                    in_=o[:, :],
                )

            # Prologue: interleaved w/x transpose pairs; the wsq reduction
            # fires per 1024-col half as soon as its 8 w-tiles are done, and
            # early h=0 main groups fill PE gaps while loads stream in.
            for g in range(MT // 2):
                transpose_pair(w_stage, wT, g, on_act=TP_SPLIT and g % 2 == 0)
                transpose_pair(x_stage, xT, g, on_act=TP_SPLIT and g % 2 == 1)
                if emit_math:
                    emit_xsq(2 * g)
                    emit_xsq(2 * g + 1)
                    if g % 4 == 3:
                        emit_wsq(g // 4)
                    if g >= 4:
                        main_group(g - 4, 0)

            if level < 3:
                continue

            # Remaining main groups: finish h=0, then h=1.
            for t in range(4, MT):
                main_group(t, 0)
            for t in range(MT):
                if LAST_SPLIT and t == MT - 1:
                    main_group(t, 1, nw=512)
                    # final 512-wide group shortens the kernel tail
                    ps_last = None  # (second 512 group below)
                    main_group_t = t
                    t1_last = None
                    # emit second half as its own 512 group
                    # (main_group with explicit offset)
                    o_h = 1
                    ps2 = mm_ps.tile([P, 512], F32, tag="mm", name="ps2")
                    for c in range(KC):
                        nc.tensor.matmul(
                            ps2[:, :],
                            xT[:, c, t * P : (t + 1) * P],
                            wT[:, c, o_h * 1024 + 512 : o_h * 1024 + 1024],
                            start=(c == 0),
                            stop=(c == KC - 1),
                        )
                    if level >= 4:
                        t1b = t1p.tile([P, 512], F32, tag="t1", name="t1b")
                        nc.vector.scalar_tensor_tensor(
                            out=t1b[:, :],
                            in0=ps2[:, :],
                            scalar=-2.0 * R512,
                            in1=wsq[:, o_h * 1024 + 512 : o_h * 1024 + 1024],
                            op0=AL.mult,
                            op1=AL.add,
                        )
                        if level >= 5:
                            ob = outp.tile([P, 512], F32, tag="o", name="ob")
                            nc.scalar.activation(
                                out=ob[:, :],
                                in_=t1b[:, :],
                                func=mybir.ActivationFunctionType.Sqrt,
                                bias=xsq_s[:, t : t + 1],
                                scale=1.0,
                            )
                            nc.sync.dma_start(
                                out=o_d[
                                    t * P : (t + 1) * P,
                                    o_h * 1024 + 512 : o_h * 1024 + 1024,
                                ],
                                in_=ob[:, :],
                            )
                else:
                    main_group(t, 1)
    nc.compile()
    return nc


_NC_CACHE = None


def _get_nc():
    global _NC_CACHE
    if _NC_CACHE is None:
        _NC_CACHE = build_nc()
    return _NC_CACHE


def kernel(x, weight):
    from concourse.bass_utils import run_bass_kernel_spmd

    x = np.ascontiguousarray(np.asarray(x, dtype=np.float32))
    weight = np.ascontiguousarray(np.asarray(weight, dtype=np.float32))
    assert x.shape == (8192, 512) and weight.shape == (4096, 512)

    nc = _get_nc()
    in_maps = []
    for c in range(8):
        bg, wg = divmod(c, 2)
        in_maps.append(
            {
                "x": x[bg * M : (bg + 1) * M],
                "w": weight[wg * N : (wg + 1) * N],
            }
        )
    res = run_bass_kernel_spmd(nc, in_maps, core_ids=list(range(8)))
    out = np.empty((8192, 4096), dtype=np.float32)
    for c in range(8):
        bg, wg = divmod(c, 2)
        out[bg * M : (bg + 1) * M, wg * N : (wg + 1) * N] = res.results[c]["out"]
    return out


# revision 29
# speedup vs baseline: 380.6821x; 1.0035x over previous
"""Trainium2 kernel for nn_ConvolutionFeatureModel.

Computes out = relu(||w_n - x_m||_2 / sqrt(512)) for x (8192, 512) and
weight (4096, 512), out (8192, 4096), all fp32.

Math:  sq_dist[m,n] = ||x_m||^2 + ||w_n||^2 - 2 x_m.w_n   (a GEMM + epilogue)
       out = sqrt(sq_dist / 512)            (relu is a no-op: sqrt >= 0)

Sharding: 8 cores as 4 batch-groups x 2 width-groups.  Per core:
x-shard (2048, 512), w-shard (2048, 512) -> out block (2048, 2048).

Per-core pipeline (Tile framework):
 - gpsimd DMA loads with fp32->bf16 cast into [128, 16, 512] stages.
 - TensorE transposes (via identity) produce xT/wT [128k, 4chunk, 2048] bf16.
 - ||w||^2/512 broadcast to all partitions via ones-matmul over squared wT.
 - ||x||^2 via ScalarE Square activation with accum_out (per-partition col).
 - Main GEMM: 8 matmuls per [128, 1024] PSUM tile (4 k-chunks x 2 n-subtiles).
 - Epilogue: DVE  t1 = psum * (-2/512) + wsq_bcast   (scalar_tensor_tensor)
             ACT  o  = Sqrt(t1 + xsq/512[bias])      (activation)
 - HWDGE (sync) DMA stores fp32 out tiles.
"""

import numpy as np

import concourse.bass as bass
import concourse.mybir as mybir
import concourse.tile as tile
from concourse import bacc
from concourse.masks import make_identity

P = 128          # partitions
K = 512          # contraction (input_dim)
KC = K // P      # 4 k-chunks
M = 2048         # batch rows per core   (8192 / 4 batch groups)
N = 2048         # width cols per core   (4096 / 2 width groups)
MT = M // P      # 16 m-tiles
NH = N // 1024   # 2 n-halves (one [128, 1024] psum tile each)
R512 = 1.0 / 512.0   # 1/SCALE^2 (SCALE = sqrt(512))

BF16 = mybir.dt.bfloat16
F32 = mybir.dt.float32

# Tunables (swept via analyze/sweep scripts)
LOAD_BOUNDS = [0, 2, 6, 11, 16]
INTERLEAVE = True      # interleave w/x transpose pairs
LAST_SPLIT = True      # split final m-tile into 512-wide groups
MM_BUFS = 3            # psum slots on the "mm" tag (2 banks each)
T1_BUFS = 6
OUT_BUFS = 6
TP_SPLIT = True        # alternate transpose-copy between DVE and ACT
XSQ_GPSIMD = False     # GPSIMD lacks accum_out on HW (walrus rejects)
SQW_GPSIMD = False     # square wT on GPSIMD instead of DVE
WHEAVY = False         # front-load w transposes (measured worse: PE ramps colder)


def build_nc(repeats=1, level=5):
    nc = bacc.Bacc("TRN2", target_bir_lowering=False)
    x_d = nc.dram_tensor("x", [M, K], F32, kind="ExternalInput")
    w_d = nc.dram_tensor("w", [N, K], F32, kind="ExternalInput")
    o_d = nc.dram_tensor("out", [M, N], F32, kind="ExternalOutput")

    AL = mybir.AluOpType
    with tile.TileContext(nc) as tc:
      for _rep in range(repeats):
        with (
            tc.tile_pool(name="big", bufs=1) as big,
            tc.tile_pool(name="mm_ps", bufs=MM_BUFS, space=bass.MemorySpace.PSUM) as mm_ps,
            tc.tile_pool(name="tp_ps", bufs=2, space=bass.MemorySpace.PSUM) as tp_ps,
            tc.tile_pool(name="sqwp", bufs=2) as sqwp,
            tc.tile_pool(name="t1p", bufs=T1_BUFS) as t1p,
            tc.tile_pool(name="outp", bufs=OUT_BUFS) as outp,
        ):
            ident = big.tile([P, P], BF16, tag="ident")
            make_identity(nc, ident[:, :])
            ones = big.tile([P, P], BF16, tag="ones")
            nc.gpsimd.memset(ones[:, :], 1.0)

            x_stage = big.tile([P, MT, K], BF16, tag="xs")   # [m_in_tile, t, k]
            w_stage = big.tile([P, MT, K], BF16, tag="ws")
            xT = big.tile([P, KC, M], BF16, tag="xT")        # [k_in_chunk, c, m]
            wT = big.tile([P, KC, N], BF16, tag="wT")
            wsq = big.tile([P, N], F32, tag="wsq")           # ||w||^2/512, bcast
            xsq_s = big.tile([P, MT], F32, tag="xsqs")       # sum x^2 / 512
            trash = big.tile([P, K], F32, tag="trash")

            x_r = x_d.rearrange("(t p) k -> p t k", p=P)
            w_r = w_d.rearrange("(t p) k -> p t k", p=P)
            # Loads with fp32->bf16 cast (SWDGE), w/x interleaved, small
            # first chunks so transposes can start early.
            if WHEAVY:
                load_sched = [
                    ("w", 0, 2), ("w", 2, 4), ("x", 0, 2), ("w", 4, 6),
                    ("x", 2, 4), ("w", 6, 8), ("x", 4, 6), ("w", 8, 16),
                    ("x", 6, 10), ("x", 10, 16),
                ]
                for which, lo, hi in load_sched:
                    st, rr = (w_stage, w_r) if which == "w" else (x_stage, x_r)
                    nc.gpsimd.dma_start(out=st[:, lo:hi, :], in_=rr[:, lo:hi, :])
            else:
                bounds = LOAD_BOUNDS
                for i in range(len(bounds) - 1):
                    lo, hi = bounds[i], bounds[i + 1]
                    nc.gpsimd.dma_start(
                        out=w_stage[:, lo:hi, :], in_=w_r[:, lo:hi, :]
                    )
                    nc.gpsimd.dma_start(
                        out=x_stage[:, lo:hi, :], in_=x_r[:, lo:hi, :]
                    )

            if level < 2:
                continue

            def transpose_pair(stage, dst, t2, on_act=False):
                """Transpose stage tiles 2*t2, 2*t2+1 into dst[:, :, 256-col slice]."""
                ps = tp_ps.tile([P, KC, 2, P], BF16, tag="tps", name="tps")
                for c in range(KC):
                    for tt in range(2):
                        t = 2 * t2 + tt
                        nc.tensor.transpose(
                            ps[:, c, tt, :],
                            stage[:, t, c * P : (c + 1) * P],
                            ident[:, :],
                        )
                dst_v = dst[:, :, 2 * t2 * P : (2 * t2 + 2) * P].rearrange(
                    "p c (tt j) -> p c tt j", j=P
                )
                if on_act:
                    nc.scalar.copy(out=dst_v, in_=ps[:, :, :, :])
                else:
                    nc.vector.tensor_copy(out=dst_v, in_=ps[:, :, :, :])

            emit_math = level >= 3

            def emit_xsq(t):
                # ||x||^2 for x tile t (square + row-sum accumulate).
                if XSQ_GPSIMD:
                    nc.gpsimd.scalar_tensor_tensor(
                        out=trash[:, :],
                        in0=x_stage[:, t, :],
                        scalar=R512,
                        in1=x_stage[:, t, :],
                        op0=AL.mult,
                        op1=AL.mult,
                        accum_out=xsq_s[:, t : t + 1],
                    )
                else:
                    nc.scalar.activation(
                        out=trash[:, :],
                        in_=x_stage[:, t, :],
                        func=mybir.ActivationFunctionType.Square,
                        scale=0.04419417382415922,
                        accum_out=xsq_s[:, t : t + 1],
                    )

            def emit_wsq(h):
                wsq_ps = mm_ps.tile([P, 1024], F32, tag="mm", name="wsq_ps")
                for c in range(KC):
                    sqw = sqwp.tile([P, 1024], BF16, tag="sqw", name="sqw")
                    eng = nc.gpsimd if SQW_GPSIMD else nc.vector
                    eng.scalar_tensor_tensor(
                        out=sqw[:, :],
                        in0=wT[:, c, h * 1024 : (h + 1) * 1024],
                        scalar=R512,
                        in1=wT[:, c, h * 1024 : (h + 1) * 1024],
                        op0=AL.mult,
                        op1=AL.mult,
                    )
                    for s in range(2):
                        nc.tensor.matmul(
                            wsq_ps[:, s * 512 : (s + 1) * 512],
                            ones[:, :],
                            sqw[:, s * 512 : (s + 1) * 512],
                            start=(c == 0),
                            stop=(c == KC - 1),
                        )
                nc.vector.tensor_copy(
                    out=wsq[:, h * 1024 : (h + 1) * 1024], in_=wsq_ps[:, :]
                )

            def main_group(t, h, nw=1024):
                """One out-tile group: GEMM accumulate + epilogue + store."""
                ps = mm_ps.tile([P, nw], F32, tag="mm", name="ps")
                for c in range(KC):
                    for s in range(nw // 512):
                        nc.tensor.matmul(
                            ps[:, s * 512 : (s + 1) * 512],
                            xT[:, c, t * P : (t + 1) * P],
                            wT[:, c, h * 1024 + s * 512 : h * 1024 + (s + 1) * 512],
                            start=(c == 0),
                            stop=(c == KC - 1),
                        )
                if level < 4:
                    return
                t1 = t1p.tile([P, nw], F32, tag="t1", name="t1")
                nc.vector.scalar_tensor_tensor(
                    out=t1[:, :],
                    in0=ps[:, :],
                    scalar=-2.0 * R512,
                    in1=wsq[:, h * 1024 : h * 1024 + nw],
                    op0=AL.mult,
                    op1=AL.add,
                )
                if level < 5:
                    return
                o = outp.tile([P, nw], F32, tag="o", name="o")
                nc.scalar.activation(
                    out=o[:, :],
                    in_=t1[:, :],
                    func=mybir.ActivationFunctionType.Sqrt,
                    bias=xsq_s[:, t : t + 1],
                    scale=1.0,
                )
                nc.sync.dma_start(
                    out=o_d[t * P : (t + 1) * P, h * 1024 : h * 1024 + nw],
                    in_=o[:, :],
                )

            # Prologue: w-heavy schedule — w pairs 0-3 first so wsq(0)
            # fires at ~8us, then x pairs interleaved with early h=0 main
            # groups filling PE gaps while the remaining loads stream in.
            if WHEAVY:
                sched = [
                    ("w", 0), ("w", 1), ("x", 0), ("w", 2), ("x", 1),
                    ("w", 3), ("W2", 0), ("x", 2), ("m", 0), ("x", 3),
                    ("m", 1), ("w", 4), ("m", 2), ("x", 4), ("m", 3),
                    ("w", 5), ("m", 4), ("x", 5), ("m", 5), ("w", 6),
                    ("m", 6), ("x", 6), ("m", 7), ("w", 7), ("W2", 1),
                    ("x", 7), ("m", 8), ("m", 9),
                ]
                na = 0
                for op, i in sched:
                    if op == "w":
                        transpose_pair(w_stage, wT, i, on_act=TP_SPLIT and na % 2 == 0)
                        na += 1
                    elif op == "x":
                        transpose_pair(x_stage, xT, i, on_act=TP_SPLIT and na % 2 == 0)
                        na += 1
                        if emit_math:
                            emit_xsq(2 * i)
                            emit_xsq(2 * i + 1)
                    elif op == "W2" and emit_math:
                        emit_wsq(i)
                    elif op == "m" and emit_math:
                        main_group(i, 0)
                first_rest = 10
            else:
                for g in range(MT // 2):
                    transpose_pair(w_stage, wT, g, on_act=TP_SPLIT and g % 2 == 0)
                    transpose_pair(x_stage, xT, g, on_act=TP_SPLIT and g % 2 == 1)
                    if emit_math:
                        emit_xsq(2 * g)
                        emit_xsq(2 * g + 1)
                        if g % 4 == 3:
                            emit_wsq(g // 4)
                        if g >= 4:
                            main_group(g - 4, 0)
                first_rest = 4

            if level < 3:
                continue

            # Remaining main groups: finish h=0, then h=1.
            for t in range(first_rest, MT):
                main_group(t, 0)
            for t in range(MT):
                if LAST_SPLIT and t >= MT - 2:
                    main_group(t, 1, nw=512)
                    # final 512-wide group shortens the kernel tail
                    ps_last = None  # (second 512 group below)
                    main_group_t = t
                    t1_last = None
                    # emit second half as its own 512 group
                    # (main_group with explicit offset)
                    o_h = 1
                    ps2 = mm_ps.tile([P, 512], F32, tag="mm", name="ps2")
                    for c in range(KC):
                        nc.tensor.matmul(
                            ps2[:, :],
                            xT[:, c, t * P : (t + 1) * P],
                            wT[:, c, o_h * 1024 + 512 : o_h * 1024 + 1024],
                            start=(c == 0),
                            stop=(c == KC - 1),
                        )
                    if level >= 4:
                        t1b = t1p.tile([P, 512], F32, tag="t1", name="t1b")
                        nc.vector.scalar_tensor_tensor(
                            out=t1b[:, :],
                            in0=ps2[:, :],
                            scalar=-2.0 * R512,
                            in1=wsq[:, o_h * 1024 + 512 : o_h * 1024 + 1024],
                            op0=AL.mult,
                            op1=AL.add,
                        )
                        if level >= 5:
                            ob = outp.tile([P, 512], F32, tag="o", name="ob")
                            nc.scalar.activation(
                                out=ob[:, :],
                                in_=t1b[:, :],
                                func=mybir.ActivationFunctionType.Sqrt,
                                bias=xsq_s[:, t : t + 1],
                                scale=1.0,
                            )
                            nc.sync.dma_start(
                                out=o_d[
                                    t * P : (t + 1) * P,
                                    o_h * 1024 + 512 : o_h * 1024 + 1024,
                                ],
                                in_=ob[:, :],
                            )
                else:
                    main_group(t, 1)
    nc.compile()
    return nc


_NC_CACHE = None


def _get_nc():
    global _NC_CACHE
    if _NC_CACHE is None:
        _NC_CACHE = build_nc()
    return _NC_CACHE


def kernel(x, weight):
    from concourse.bass_utils import run_bass_kernel_spmd

    x = np.ascontiguousarray(np.asarray(x, dtype=np.float32))
    weight = np.ascontiguousarray(np.asarray(weight, dtype=np.float32))
    assert x.shape == (8192, 512) and weight.shape == (4096, 512)

    nc = _get_nc()
    in_maps = []
    for c in range(8):
        bg, wg = divmod(c, 2)
        in_maps.append(
            {
                "x": x[bg * M : (bg + 1) * M],
                "w": weight[wg * N : (wg + 1) * N],
            }
        )
    res = run_bass_kernel_spmd(nc, in_maps, core_ids=list(range(8)))
    out = np.empty((8192, 4096), dtype=np.float32)
    for c in range(8):
        bg, wg = divmod(c, 2)
        out[bg * M : (bg + 1) * M, wg * N : (wg + 1) * N] = res.results[c]["out"]
    return out


# revision 31
# speedup vs baseline: 387.4522x; 1.0178x over previous
"""Trainium2 kernel for nn_ConvolutionFeatureModel.

Computes out = relu(||w_n - x_m||_2 / sqrt(512)) for x (8192, 512) and
weight (4096, 512), out (8192, 4096), all fp32.

Math:  sq_dist[m,n] = ||x_m||^2 + ||w_n||^2 - 2 x_m.w_n   (a GEMM + epilogue)
       out = sqrt(sq_dist / 512)            (relu is a no-op: sqrt >= 0)

Sharding: 8 cores as 4 batch-groups x 2 width-groups.  Per core:
x-shard (2048, 512), w-shard (2048, 512) -> out block (2048, 2048).

Per-core pipeline (Tile framework):
 - gpsimd DMA loads with fp32->bf16 cast into [128, 16, 512] stages.
 - TensorE transposes (via identity) produce xT/wT [128k, 4chunk, 2048] bf16.
 - ||w||^2/512 broadcast to all partitions via ones-matmul over squared wT.
 - ||x||^2 via ScalarE Square activation with accum_out (per-partition col).
 - Main GEMM: 8 matmuls per [128, 1024] PSUM tile (4 k-chunks x 2 n-subtiles).
 - Epilogue: DVE  t1 = psum * (-2/512) + wsq_bcast   (scalar_tensor_tensor)
             ACT  o  = Sqrt(t1 + xsq/512[bias])      (activation)
 - HWDGE (sync) DMA stores fp32 out tiles.
"""

import numpy as np

import concourse.bass as bass
import concourse.mybir as mybir
import concourse.tile as tile
from concourse import bacc
from concourse.masks import make_identity

P = 128          # partitions
K = 512          # contraction (input_dim)
KC = K // P      # 4 k-chunks
M = 2048         # batch rows per core   (8192 / 4 batch groups)
N = 2048         # width cols per core   (4096 / 2 width groups)
MT = M // P      # 16 m-tiles
NH = N // 1024   # 2 n-halves (one [128, 1024] psum tile each)
R512 = 1.0 / 512.0   # 1/SCALE^2 (SCALE = sqrt(512))

BF16 = mybir.dt.bfloat16
F32 = mybir.dt.float32

# Tunables (swept via analyze/sweep scripts)
LOAD_BOUNDS = [0, 2, 6, 11, 16]
INTERLEAVE = True      # interleave w/x transpose pairs
LAST_SPLIT = True      # split final m-tile into 512-wide groups
MM_BUFS = 3            # psum slots on the "mm" tag (2 banks each)
T1_BUFS = 6
OUT_BUFS = 6
TP_SPLIT = True        # alternate transpose-copy between DVE and ACT
XSQ_GPSIMD = False     # GPSIMD lacks accum_out on HW (walrus rejects)
SQW_GPSIMD = False     # square wT on GPSIMD instead of DVE
WHEAVY = False         # front-load w transposes (measured worse: PE ramps colder)
SQW_BUFS = 2           # sqw staging depth
SPLIT_FROM = 8         # last SPLIT_FROM m-tiles get 512-wide epilogue groups


def build_nc(repeats=1, level=5):
    nc = bacc.Bacc("TRN2", target_bir_lowering=False)
    x_d = nc.dram_tensor("x", [M, K], F32, kind="ExternalInput")
    w_d = nc.dram_tensor("w", [N, K], F32, kind="ExternalInput")
    o_d = nc.dram_tensor("out", [M, N], F32, kind="ExternalOutput")

    AL = mybir.AluOpType
    with tile.TileContext(nc) as tc:
      for _rep in range(repeats):
        with (
            tc.tile_pool(name="big", bufs=1) as big,
            tc.tile_pool(name="mm_ps", bufs=MM_BUFS, space=bass.MemorySpace.PSUM) as mm_ps,
            tc.tile_pool(name="tp_ps", bufs=2, space=bass.MemorySpace.PSUM) as tp_ps,
            tc.tile_pool(name="sqwp", bufs=SQW_BUFS) as sqwp,
            tc.tile_pool(name="t1p", bufs=T1_BUFS) as t1p,
            tc.tile_pool(name="outp", bufs=OUT_BUFS) as outp,
        ):
            ident = big.tile([P, P], BF16, tag="ident")
            make_identity(nc, ident[:, :])
            ones = big.tile([P, P], BF16, tag="ones")
            nc.gpsimd.memset(ones[:, :], 1.0)

            x_stage = big.tile([P, MT, K], BF16, tag="xs")   # [m_in_tile, t, k]
            w_stage = big.tile([P, MT, K], BF16, tag="ws")
            xT = big.tile([P, KC, M], BF16, tag="xT")        # [k_in_chunk, c, m]
            wT = big.tile([P, KC, N], BF16, tag="wT")
            wsq = big.tile([P, N], F32, tag="wsq")           # ||w||^2/512, bcast
            xsq_s = big.tile([P, MT], F32, tag="xsqs")       # sum x^2 / 512
            trash = big.tile([P, K], F32, tag="trash")

            x_r = x_d.rearrange("(t p) k -> p t k", p=P)
            w_r = w_d.rearrange("(t p) k -> p t k", p=P)
            # Loads with fp32->bf16 cast (SWDGE), w/x interleaved, small
            # first chunks so transposes can start early.
            if WHEAVY:
                load_sched = [
                    ("w", 0, 2), ("w", 2, 4), ("x", 0, 2), ("w", 4, 6),
                    ("x", 2, 4), ("w", 6, 8), ("x", 4, 6), ("w", 8, 16),
                    ("x", 6, 10), ("x", 10, 16),
                ]
                for which, lo, hi in load_sched:
                    st, rr = (w_stage, w_r) if which == "w" else (x_stage, x_r)
                    nc.gpsimd.dma_start(out=st[:, lo:hi, :], in_=rr[:, lo:hi, :])
            else:
                bounds = LOAD_BOUNDS
                for i in range(len(bounds) - 1):
                    lo, hi = bounds[i], bounds[i + 1]
                    nc.gpsimd.dma_start(
                        out=w_stage[:, lo:hi, :], in_=w_r[:, lo:hi, :]
                    )
                    nc.gpsimd.dma_start(
                        out=x_stage[:, lo:hi, :], in_=x_r[:, lo:hi, :]
                    )

            if level < 2:
                continue

            def transpose_pair(stage, dst, t2, on_act=False):
                """Transpose stage tiles 2*t2, 2*t2+1 into dst[:, :, 256-col slice]."""
                ps = tp_ps.tile([P, KC, 2, P], BF16, tag="tps", name="tps")
                for c in range(KC):
                    for tt in range(2):
                        t = 2 * t2 + tt
                        nc.tensor.transpose(
                            ps[:, c, tt, :],
                            stage[:, t, c * P : (c + 1) * P],
                            ident[:, :],
                        )
                dst_v = dst[:, :, 2 * t2 * P : (2 * t2 + 2) * P].rearrange(
                    "p c (tt j) -> p c tt j", j=P
                )
                if on_act:
                    nc.scalar.copy(out=dst_v, in_=ps[:, :, :, :])
                else:
                    nc.vector.tensor_copy(out=dst_v, in_=ps[:, :, :, :])

            emit_math = level >= 3

            def emit_xsq(t):
                # ||x||^2 for x tile t (square + row-sum accumulate).
                if XSQ_GPSIMD:
                    nc.gpsimd.scalar_tensor_tensor(
                        out=trash[:, :],
                        in0=x_stage[:, t, :],
                        scalar=R512,
                        in1=x_stage[:, t, :],
                        op0=AL.mult,
                        op1=AL.mult,
                        accum_out=xsq_s[:, t : t + 1],
                    )
                else:
                    nc.scalar.activation(
                        out=trash[:, :],
                        in_=x_stage[:, t, :],
                        func=mybir.ActivationFunctionType.Square,
                        scale=0.04419417382415922,
                        accum_out=xsq_s[:, t : t + 1],
                    )

            def emit_wsq(h):
                wsq_ps = mm_ps.tile([P, 1024], F32, tag="mm", name="wsq_ps")
                for c in range(KC):
                    sqw = sqwp.tile([P, 1024], BF16, tag="sqw", name="sqw")
                    eng = nc.gpsimd if SQW_GPSIMD else nc.vector
                    eng.scalar_tensor_tensor(
                        out=sqw[:, :],
                        in0=wT[:, c, h * 1024 : (h + 1) * 1024],
                        scalar=R512,
                        in1=wT[:, c, h * 1024 : (h + 1) * 1024],
                        op0=AL.mult,
                        op1=AL.mult,
                    )
                    for s in range(2):
                        nc.tensor.matmul(
                            wsq_ps[:, s * 512 : (s + 1) * 512],
                            ones[:, :],
                            sqw[:, s * 512 : (s + 1) * 512],
                            start=(c == 0),
                            stop=(c == KC - 1),
                        )
                nc.vector.tensor_copy(
                    out=wsq[:, h * 1024 : (h + 1) * 1024], in_=wsq_ps[:, :]
                )

            def main_group(t, h, nw=1024):
                """One out-tile group: GEMM accumulate + epilogue + store."""
                ps = mm_ps.tile([P, nw], F32, tag="mm", name="ps")
                for c in range(KC):
                    for s in range(nw // 512):
                        nc.tensor.matmul(
                            ps[:, s * 512 : (s + 1) * 512],
                            xT[:, c, t * P : (t + 1) * P],
                            wT[:, c, h * 1024 + s * 512 : h * 1024 + (s + 1) * 512],
                            start=(c == 0),
                            stop=(c == KC - 1),
                        )
                if level < 4:
                    return
                t1 = t1p.tile([P, nw], F32, tag="t1", name="t1")
                nc.vector.scalar_tensor_tensor(
                    out=t1[:, :],
                    in0=ps[:, :],
                    scalar=-2.0 * R512,
                    in1=wsq[:, h * 1024 : h * 1024 + nw],
                    op0=AL.mult,
                    op1=AL.add,
                )
                if level < 5:
                    return
                o = outp.tile([P, nw], F32, tag="o", name="o")
                nc.scalar.activation(
                    out=o[:, :],
                    in_=t1[:, :],
                    func=mybir.ActivationFunctionType.Sqrt,
                    bias=xsq_s[:, t : t + 1],
                    scale=1.0,
                )
                nc.sync.dma_start(
                    out=o_d[t * P : (t + 1) * P, h * 1024 : h * 1024 + nw],
                    in_=o[:, :],
                )

            # Prologue: w-heavy schedule — w pairs 0-3 first so wsq(0)
            # fires at ~8us, then x pairs interleaved with early h=0 main
            # groups filling PE gaps while the remaining loads stream in.
            if WHEAVY:
                sched = [
                    ("w", 0), ("w", 1), ("x", 0), ("w", 2), ("x", 1),
                    ("w", 3), ("W2", 0), ("x", 2), ("m", 0), ("x", 3),
                    ("m", 1), ("w", 4), ("m", 2), ("x", 4), ("m", 3),
                    ("w", 5), ("m", 4), ("x", 5), ("m", 5), ("w", 6),
                    ("m", 6), ("x", 6), ("m", 7), ("w", 7), ("W2", 1),
                    ("x", 7), ("m", 8), ("m", 9),
                ]
                na = 0
                for op, i in sched:
                    if op == "w":
                        transpose_pair(w_stage, wT, i, on_act=TP_SPLIT and na % 2 == 0)
                        na += 1
                    elif op == "x":
                        transpose_pair(x_stage, xT, i, on_act=TP_SPLIT and na % 2 == 0)
                        na += 1
                        if emit_math:
                            emit_xsq(2 * i)
                            emit_xsq(2 * i + 1)
                    elif op == "W2" and emit_math:
                        emit_wsq(i)
                    elif op == "m" and emit_math:
                        main_group(i, 0)
                first_rest = 10
            else:
                for g in range(MT // 2):
                    transpose_pair(w_stage, wT, g, on_act=TP_SPLIT and g % 2 == 0)
                    transpose_pair(x_stage, xT, g, on_act=TP_SPLIT and g % 2 == 1)
                    if emit_math:
                        emit_xsq(2 * g)
                        emit_xsq(2 * g + 1)
                        if g % 4 == 3:
                            emit_wsq(g // 4)
                        if g >= 4:
                            main_group(g - 4, 0)
                first_rest = 4

            if level < 3:
                continue

            # Remaining main groups: finish h=0, then h=1.
            for t in range(first_rest, MT):
                main_group(t, 0)
            for t in range(MT):
                if LAST_SPLIT and t >= MT - SPLIT_FROM:
                    main_group(t, 1, nw=512)
                    # final 512-wide group shortens the kernel tail
                    ps_last = None  # (second 512 group below)
                    main_group_t = t
                    t1_last = None
                    # emit second half as its own 512 group
                    # (main_group with explicit offset)
                    o_h = 1
                    ps2 = mm_ps.tile([P, 512], F32, tag="mm", name="ps2")
                    for c in range(KC):
                        nc.tensor.matmul(
                            ps2[:, :],
                            xT[:, c, t * P : (t + 1) * P],
                            wT[:, c, o_h * 1024 + 512 : o_h * 1024 + 1024],
                            start=(c == 0),
                            stop=(c == KC - 1),
                        )
                    if level >= 4:
                        t1b = t1p.tile([P, 512], F32, tag="t1", name="t1b")
                        nc.vector.scalar_tensor_tensor(
                            out=t1b[:, :],
                            in0=ps2[:, :],
                            scalar=-2.0 * R512,
                            in1=wsq[:, o_h * 1024 + 512 : o_h * 1024 + 1024],
                            op0=AL.mult,
                            op1=AL.add,
                        )
                        if level >= 5:
                            ob = outp.tile([P, 512], F32, tag="o", name="ob")
                            nc.scalar.activation(
                                out=ob[:, :],
                                in_=t1b[:, :],
                                func=mybir.ActivationFunctionType.Sqrt,
                                bias=xsq_s[:, t : t + 1],
                                scale=1.0,
                            )
                            nc.sync.dma_start(
                                out=o_d[
                                    t * P : (t + 1) * P,
                                    o_h * 1024 + 512 : o_h * 1024 + 1024,
                                ],
                                in_=ob[:, :],
                            )
                else:
                    main_group(t, 1)
    nc.compile()
    return nc


_NC_CACHE = None


def _get_nc():
    global _NC_CACHE
    if _NC_CACHE is None:
        _NC_CACHE = build_nc()
    return _NC_CACHE


def kernel(x, weight):
    from concourse.bass_utils import run_bass_kernel_spmd

    x = np.ascontiguousarray(np.asarray(x, dtype=np.float32))
    weight = np.ascontiguousarray(np.asarray(weight, dtype=np.float32))
    assert x.shape == (8192, 512) and weight.shape == (4096, 512)

    nc = _get_nc()
    in_maps = []
    for c in range(8):
        bg, wg = divmod(c, 2)
        in_maps.append(
            {
                "x": x[bg * M : (bg + 1) * M],
                "w": weight[wg * N : (wg + 1) * N],
            }
        )
    res = run_bass_kernel_spmd(nc, in_maps, core_ids=list(range(8)))
    out = np.empty((8192, 4096), dtype=np.float32)
    for c in range(8):
        bg, wg = divmod(c, 2)
        out[bg * M : (bg + 1) * M, wg * N : (wg + 1) * N] = res.results[c]["out"]
    return out
